# revision 29
# baseline (speedup 1.0000x reference)
"""Causal self-attention (B=2, T=4096, C=768, H=12, D=64) on 8 trn2 cores.

Sharding: core c handles batch b = c//4 and heads [3g, 3g+3), g = c%4.
Each core computes a (4096, 768) partial of y = attn_out @ w_out restricted
to its 3 heads' rows of w_out; the host sums the 4 partials per batch.

v1 layout (vs baseline): x arrives host-transposed (xT [C, T]) so no PE
transposes are needed; V is projected token-major directly (stationary =
xT chunk, moving = w_v), QK uses feature-major Q^T/K^T from 3 projection
slots [q0|q1], [k0|k1], [q2|k2] (k2 partition-shifted via SBUF DMA).
Causal masking touches only the [128,128] triangle block per diagonal
k-tile (gpsimd affine_select); the fully-masked columns are skipped by
column-restricted exp + PV accumulation. PV of group g is issued after
the QK+exp of group g+1 so the scalar engine (exp is the global floor,
~1.57us per k-tile) never starves.

Math per head (no max-subtraction softmax; scores are O(8) so exp is safe):
  S^T[k, q] = (K Q^T)[k, q] / 8     computed k-on-partitions (f32r matmuls)
  E = exp(S^T) * causal_mask
  [Y^T; l] = [V | 1]^T E            PV matmul with a ones column -> row 64 = l
  out += (Y^T / l).T @ W_o[head rows]
"""

import os
import numpy as np
import ml_dtypes
from contextlib import ExitStack

import concourse.bass as bass
import concourse.tile as tile
from concourse import bacc, mybir
from concourse.bass_utils import run_bass_kernel_spmd

F32 = mybir.dt.float32
BF16 = mybir.dt.bfloat16

B, T, C, H, D = 2, 4096, 768, 12, 64
HPC = 3            # heads per core
NS = 8             # strips
SW = 512           # strip width (q)
KT = 128           # k tile
NKT = T // KT      # 32 k tiles
KG = 8             # k tiles per PV accumulation group





def build_program():
    nc = bacc.Bacc("TRN2", target_bir_lowering=False, debug=False, num_devices=8)

    x_d = nc.dram_tensor("xT", [C, T], BF16, kind="ExternalInput").ap()
    wqk_d = nc.dram_tensor("wqk", [C, 384], BF16, kind="ExternalInput").ap()
    wv_d = nc.dram_tensor("wv", [C, 192], BF16, kind="ExternalInput").ap()
    woA_d = nc.dram_tensor("woA", [128, C], BF16, kind="ExternalInput").ap()
    woB_d = nc.dram_tensor("woB", [64, C], BF16, kind="ExternalInput").ap()
    y_d = nc.dram_tensor("y", [T, C], BF16, kind="ExternalOutput").ap()

    with tile.TileContext(nc) as tc, ExitStack() as ctx:
        kernel_body(tc, ctx, x_d, wqk_d, wv_d, woA_d, woB_d, y_d)
    nc.compile()
    return nc


def kernel_body(tc, ctx, x_d, wqk_d, wv_d, woA_d, woB_d, y_d):
    nc = tc.nc
    EXP = mybir.ActivationFunctionType.Exp
    k_diag = int(os.environ.get("KDIAG", "1"))   # col-restricted diag PV
    k_pb = int(os.environ.get("KPB", "0"))       # gpsimd partition_broadcast
    k_rf = int(os.environ.get("KRF", "0"))       # reciprocal_approx_fast
    k_warm = int(os.environ.get("KWARM", "1"))   # PE warmup matmuls
    k_keep = int(os.environ.get("KKEEP", "0"))   # HAM keepalive zero-MMs/tick
    k_keepfrom = int(os.environ.get("KKEEPFROM", "4"))
    dram_pool = ctx.enter_context(tc.tile_pool(name="dram", bufs=1, space="DRAM"))
    scratch_d = dram_pool.tile([NS, HPC, SW], F32, name="scratch")

    singles = ctx.enter_context(tc.tile_pool(name="singles", bufs=1))
    xt_pool = ctx.enter_context(tc.tile_pool(name="xt_pool", bufs=4))
    qq_pool = ctx.enter_context(tc.tile_pool(name="qq_pool", bufs=3))
    es_pool = ctx.enter_context(tc.tile_pool(name="es_pool", bufs=20))
    ya_pool = ctx.enter_context(tc.tile_pool(name="ya_pool", bufs=2))
    rl_pool = ctx.enter_context(tc.tile_pool(name="rl_pool", bufs=2))
    yst_pool = ctx.enter_context(tc.tile_pool(name="yst_pool", bufs=2))
    out_pool = ctx.enter_context(tc.tile_pool(name="out_pool", bufs=2))
    ps_s = ctx.enter_context(tc.tile_pool(name="ps_s", bufs=2, space="PSUM"))
    ps_y = ctx.enter_context(tc.tile_pool(name="ps_y", bufs=2, space="PSUM"))

    # ---- PE warmup: junk matmuls during the initial DMA wait keep HAM hot ----
    junk = singles.tile([128, 128], BF16)
    nc.vector.memset(junk, 0.015625)
    # zero stationary + junk moving: accumulating 0 into an open PV psum
    # tile is a numerically-free PE keepalive (HAM stays at K=8/8)
    junkz = singles.tile([128, D + 1], BF16)
    nc.vector.memset(junkz, 0.0)
    junkr = singles.tile([128, SW], BF16)
    nc.vector.memset(junkr, 0.0)
    ones64 = singles.tile([1, 64], BF16)
    nc.vector.memset(ones64, 1.0)
    if k_warm:
        psj = ps_y.tile([128, SW], F32, name="ps_warm", tag="psy")
        for w in range(32):
            nc.tensor.matmul(psj[:, 0:128], (junk), (junk),
                             start=True, stop=True)

    # ---- weights (xT strip 0 + wqk first: they gate the first QK) ----
    xT_tiles = [None] * NS
    x_r = x_d.rearrange("(kc p) t -> p kc t", kc=6)
    xt0 = xt_pool.tile([128, 6, SW], BF16, name="xT_0", tag="xT")
    for kc in range(6):
        eng = nc.sync if kc % 2 == 0 else nc.gpsimd
        eng.dma_start(xt0[:, kc, :], x_r[:, kc, 0:SW])
    xT_tiles[0] = xt0
    wqk_sb = singles.tile([128, 6, 384], BF16, name="wqk_sb")
    nc.sync.dma_start(wqk_sb, wqk_d.rearrange("(kc p) f -> p kc f", kc=6))
    wv_sb = singles.tile([128, 6, 192], BF16, name="wv_sb")
    nc.gpsimd.dma_start(wv_sb, wv_d.rearrange("(kc p) f -> p kc f", kc=6))
    woA = singles.tile([128, C], BF16)
    woB = singles.tile([64, C], BF16)

    # resident K storage: KK[s] = [k0|k1] feature-major, K2c[s] = k2 at p0:64
    KK = [singles.tile([128, SW], BF16, name=f"KK{s}") for s in range(NS)]
    K2c = [singles.tile([64, SW], BF16, name=f"K2c{s}") for s in range(NS)]

    # token-major V with ones column per head, all 32 k-tiles
    vtm = [singles.tile([128, NKT, D + 1], BF16, name=f"vtm{h}") for h in range(HPC)]
    ones_col = singles.tile([128, NKT], BF16)
    nc.vector.memset(ones_col, 1.0)
    for h in range(HPC):
        nc.vector.tensor_copy(vtm[h][:, :, D:D + 1], ones_col.unsqueeze(2))

    qq_tiles = [None] * NS

    # ---------------- Phase A for one strip (chunk generator) ----------------
    def phase_a_dma(s):
        xt = xt_pool.tile([128, 6, SW], BF16, name=f"xT_{s}", tag="xT")
        for kc in range(6):
            eng = nc.sync if kc % 2 == 0 else nc.gpsimd
            eng.dma_start(xt[:, kc, :], x_r[:, kc, s * SW:(s + 1) * SW])
        xT_tiles[s] = xt

    def phase_a_proj(s):
        xT = xT_tiles[s]
        # projection slots: [q0|q1], [k0|k1], [q2|k2] -- each slot gets its
        # own 1-bank psum tile from the psy ring so the QK double-buffer
        # ("S" ring) is never starved by projection work. Slot 2 first: its
        # k2 partition-shift DMA is on the critical path to the first QK.
        qq = tmp = None
        for u in (2, 0, 1):
            psp = ps_y.tile([128, SW], F32, name=f"ps_pj_{s}_{u}", tag="psy")
            for kc in range(6):
                nc.tensor.matmul(psp,
                                 (wqk_sb[:, kc, u * 128:(u + 1) * 128]),
                                 (xT[:, kc, :]), start=(kc == 0), stop=(kc == 5))
            if u == 0:
                qq = qq_pool.tile([128, SW], BF16, name=f"qq_{s}", tag="qq")
                nc.vector.tensor_copy(qq, psp)
                qq_tiles[s] = (qq, tmp)
            elif u == 1:
                nc.vector.tensor_copy(KK[s], psp)
            else:
                tmp = qq_pool.tile([128, SW], BF16, name=f"q2k2_{s}",
                                   tag="q2k2")
                nc.vector.tensor_copy(tmp, psp)
                # k2 partition shift p64:128 -> p0:64 (SBUF->SBUF DMA)
                nc.gpsimd.dma_start(K2c[s], tmp[64:128, :])
            yield

    def phase_a_v(s):
        # V token-major: stationary = xT chunk slice, moving = w_v [128, 192]
        # (pure filler: vtm k-tiles of strip s are first read by strip s's
        # diagonal PV unit, which issues at the end of strip s)
        xT = xT_tiles[s]
        for tt in range(4):
            psv = ps_y.tile([128, 192], F32, name=f"ps_v_{s}_{tt}", tag="psy")
            for kc in range(6):
                nc.tensor.matmul(psv,
                                 (xT[:, kc, tt * 128:(tt + 1) * 128]),
                                 (wv_sb[:, kc, :]), start=(kc == 0), stop=(kc == 5))
            kt = 4 * s + tt
            for h in range(HPC):
                nc.vector.tensor_copy(vtm[h][:, kt, 0:D],
                                      psv[:, h * 64:(h + 1) * 64])
            yield

    # -------- Phase B: one continuous pipeline over all 144 k-tiles --------
    # Per tick (one k-tile): QK triplet + exp + ~3 PV matmuls from the unit
    # queue (one (strip, group, head) unit at a time, so only 1-2 psy banks
    # are ever live) + one filler chunk. PV lags its group's last exp by
    # >= 2 ticks so the PE FIFO never blocks on the scalar engine.
    fillers = []
    pa_gens = {}

    def fill_one():
        while fillers:
            g = fillers.pop(0)
            try:
                next(g)
            except StopIteration:
                continue
            fillers.append(g)
            return True
        return False

    yaccs = {}

    open_psy = {"t": None}

    def make_unit(s, gi, grp, h, es_grp):
        """Returns list of thunks: 8 (or 4) PV matmuls then the yacc flush."""
        psy_box = {}

        def mm(u, i):
            def run():
                if u == 0:
                    psy_box["t"] = ps_y.tile([65, SW], F32,
                                             name=f"ps_y_{s}_{gi}_{h}", tag="psy")
                    open_psy["t"] = psy_box["t"]
                psy = psy_box["t"]
                es = es_grp[i]
                o = i - 4 * s
                last = len(grp) - 1
                if k_diag and o > 0:
                    nc.tensor.matmul(psy[:, 128 * o:], (vtm[h][:, i, :]),
                                     (es[:, h, 128 * o:]),
                                     start=False, stop=(u == last))
                else:
                    nc.tensor.matmul(psy, (vtm[h][:, i, :]), (es[:, h, :]),
                                     start=(u == 0), stop=(u == last))
                if u == last and open_psy["t"] is psy:
                    open_psy["t"] = None
            return run

        def flush():
            psy = psy_box["t"]
            if open_psy["t"] is psy:
                open_psy["t"] = None
            if gi == 0:
                nc.vector.tensor_copy(yaccs[s][h], psy)
            else:
                nc.vector.tensor_add(yaccs[s][h], yaccs[s][h], psy)

        thunks = [mm(u, i) for u, i in enumerate(grp)]
        thunks.append(flush)
        return thunks

    # PV work queue: per tick pop up to 3 thunks whose eligibility tick passed
    pvq = []          # list of (eligible_tick, thunk)

    def pump_pv(tick, n=3):
        done = 0
        while pvq and done < n:
            et, th = pvq[0]
            if et > tick:
                break
            pvq.pop(0)
            th()
            if th.__name__ != "flush":
                done += 1

    def run_pipeline():
        tick = 0
        for s in range(NS):
            nkt = 4 * s + 4
            if s == 1:
                nc.sync.dma_start(woA, woA_d)
                nc.sync.dma_start(woB, woB_d)
            if s + 2 < NS:
                phase_a_dma(s + 2)
            if s + 1 < NS:
                g = phase_a_proj(s + 1)
                pa_gens[s + 1] = g
                fillers.append(g)
            if s >= 1:
                fillers.append(phase_a_v(s))
            if s == 1:
                for _ in phase_a_v(0):
                    pass
            # ensure this strip's projections are fully issued
            g = pa_gens.get(s)
            if g is not None:
                for _ in g:
                    pass
            qq, tmp = qq_tiles[s]
            qq2 = tmp[0:64, :]
            yaccs[s] = [ya_pool.tile([65, SW], F32, name=f"yacc_{s}_{h}",
                                     tag=f"yacc{h}") for h in range(HPC)]

            groups = [list(range(gg, min(gg + KG, nkt)))
                      for gg in range(0, nkt, KG)]
            es_grp = {}
            for gi, grp in enumerate(groups):
                for u, i in enumerate(grp):
                    pss = ps_s.tile([128, 3, SW], F32,
                                    name=f"ps_s_{s}_{i}", tag="S")
                    st = KK[i // 4]
                    sl = slice((i % 4) * 128, (i % 4) * 128 + 128)
                    nc.tensor.matmul(pss[:, 0, :], (st[0:64, sl]),
                                     (qq[0:64, :]), start=True, stop=True)
                    nc.tensor.matmul(pss[:, 1, :], (st[64:128, sl]),
                                     (qq[64:128, :]), start=True, stop=True)
                    nc.tensor.matmul(pss[:, 2, :], (K2c[i // 4][:, sl]),
                                     (qq2), start=True, stop=True)
                    es = es_pool.tile([128, 3, SW], BF16,
                                      name=f"es_{s}_{i}", tag="es")
                    o = i - 4 * s
                    if o < 0:
                        nc.scalar.activation(es, pss, EXP, scale=0.125)
                    else:
                        nc.scalar.activation(es[:, :, 128 * o:],
                                             pss[:, :, 128 * o:],
                                             EXP, scale=0.125)
                        for h in range(HPC):
                            blk = es[:, h, 128 * o:128 * (o + 1)]
                            nc.gpsimd.affine_select(
                                out=blk, in_=blk,
                                compare_op=mybir.AluOpType.is_ge, fill=0.0,
                                base=0, pattern=[[1, 128]],
                                channel_multiplier=-1)
                        if not k_diag and o > 0:
                            nc.gpsimd.memset(es[:, :, 0:128 * o], 0.0)
                    es_grp[i] = es
                    pump_pv(tick, n=4 if s == NS - 1 else 3)
                    if k_keep and s >= k_keepfrom and open_psy["t"] is not None:
                        for _ in range(k_keep):
                            nc.tensor.matmul(open_psy["t"], (junkz), (junkr),
                                             start=False, stop=False)
                    fill_one()
                    if 1 <= s <= 2:
                        fill_one()
                    tick += 1
                # group's exps all issued: enqueue its 3 PV units
                et = tick + 1
                for h in range(HPC):
                    for th in make_unit(s, gi, grp, h, dict(es_grp)):
                        pvq.append((et, th))
            # strip done: schedule epilogue after its last units complete
            if s > 0:
                fillers.append(epilogue(s - 1, yaccs[s - 1]))
        # drain
        while pvq:
            et, th = pvq.pop(0)
            th()
            fill_one()
        for _ in epilogue(NS - 1, yaccs[NS - 1]):
            pass
        while fill_one():
            pass

    # ---- strip epilogue: normalize + output projection (deferred) ----
    # Fully chunked per 128 q-columns: reciprocal, DRAM-bounce broadcast,
    # normalize and output projection pipeline across DVE/DMA/PE so the
    # final strip's tail is short.
    def epilogue(s, yacc):
        lrow = rl_pool.tile([3, SW], F32, name=f"lrow_{s}", tag="lrow")
        for h in range(HPC):
            eng = nc.sync if h == 0 else nc.gpsimd
            eng.dma_start(lrow[h:h + 1, :], yacc[h][64:65, :])
        yield
        for ch in range(4):
            csl = slice(ch * 128, (ch + 1) * 128)
            nc.vector.reciprocal(lrow[:, csl], lrow[:, csl])
            nc.sync.dma_start(scratch_d[s, :, csl], lrow[:, csl])
            yield

        ya = yst_pool.tile([128, SW], BF16, name=f"ya_{s}", tag="ya")
        y2 = yst_pool.tile([64, SW], BF16, name=f"y2_{s}", tag="y2")
        ytmp = yst_pool.tile([64, SW], BF16, name=f"ytmp_{s}", tag="ytmp")
        for ch in range(4):
            csl = slice(ch * 128, (ch + 1) * 128)
            rb = rl_pool.tile([64, 3, 128], F32, name=f"rlb_{s}_{ch}",
                              tag="rlb")
            nc.gpsimd.dma_start(
                rb, scratch_d[s, :, csl].unsqueeze(0).to_broadcast((64, 3, 128)))
            nc.vector.tensor_mul(ya[0:64, csl], yacc[0][0:64, csl], rb[:, 0, :])
            nc.vector.tensor_mul(ytmp[:, csl], yacc[1][0:64, csl], rb[:, 1, :])
            nc.vector.tensor_mul(y2[:, csl], yacc[2][0:64, csl], rb[:, 2, :])
            nc.gpsimd.dma_start(ya[64:128, csl], ytmp[:, csl])
            yield

            # out projection for this 128-q tile
            qsl = csl
            osb = out_pool.tile([128, C], BF16, name=f"osb_{s}_{ch}", tag="osb")
            for (n0, n1) in ((0, 512), (512, 768)):
                pso = ps_y.tile([128, n1 - n0], F32,
                                name=f"ps_o_{s}_{ch}_{n0}", tag="psy")
                nc.tensor.matmul(pso, (ya[:, qsl]),
                                 (woA[:, n0:n1]), start=True, stop=False)
                nc.tensor.matmul(pso, (y2[:, qsl]),
                                 (woB[:, n0:n1]), start=False, stop=True)
                nc.vector.tensor_copy(osb[:, n0:n1], pso)
            nc.sync.dma_start(y_d[s * SW + ch * 128: s * SW + (ch + 1) * 128, :],
                              osb)
            if ch < 3:
                yield

    for _ in phase_a_proj(0):
        pass
    phase_a_dma(1)
    run_pipeline()


_PROGRAM_CACHE = {}


def _get_program():
    if "nc" not in _PROGRAM_CACHE:
        _PROGRAM_CACHE["nc"] = build_program()
    return _PROGRAM_CACHE["nc"]


def make_in_maps(x, w_qkv, w_out):
    x = np.asarray(x, dtype=np.float32)
    w_qkv = np.asarray(w_qkv, dtype=np.float32)
    w_out = np.asarray(w_out, dtype=np.float32)
    in_maps = []
    for c in range(8):
        b, g = c // 4, c % 4
        base = 192 * g
        q01 = w_qkv[:, base:base + 128]
        q2 = w_qkv[:, base + 128:base + 192]
        k01 = w_qkv[:, 768 + base:768 + base + 128]
        k2 = w_qkv[:, 768 + base + 128:768 + base + 192]
        wqk = np.concatenate([q01, k01, q2, k2], axis=1)
        wv = w_qkv[:, 1536 + base:1536 + base + 192]
        bf = ml_dtypes.bfloat16
        in_maps.append({
            "xT": np.ascontiguousarray(x[b].T.astype(bf)),
            "wqk": np.ascontiguousarray(wqk.astype(bf)),
            "wv": np.ascontiguousarray(wv.astype(bf)),
            "woA": np.ascontiguousarray(w_out[base:base + 128].astype(bf)),
            "woB": np.ascontiguousarray(w_out[base + 128:base + 192].astype(bf)),
        })
    return in_maps


def kernel(x, w_qkv, w_out, trace=False):
    nc = _get_program()
    in_maps = make_in_maps(x, w_qkv, w_out)
    res = run_bass_kernel_spmd(nc, in_maps, list(range(8)), trace=trace)
    out = np.zeros((B, T, C), dtype=np.float32)
    for c in range(8):
        out[c // 4] += np.asarray(res.results[c]["y"], dtype=np.float32)
    kernel.last_result = res
    return out


# revision 30
# speedup vs baseline: 1.0256x; 1.0256x over previous
"""Causal self-attention (B=2, T=4096, C=768, H=12, D=64) on 8 trn2 cores.

Sharding: core c handles batch b = c//4 and heads [3g, 3g+3), g = c%4.
Each core computes a (4096, 768) partial of y = attn_out @ w_out restricted
to its 3 heads' rows of w_out; the host sums the 4 partials per batch.

v1 layout (vs baseline): x arrives host-transposed (xT [C, T]) so no PE
transposes are needed; V is projected token-major directly (stationary =
xT chunk, moving = w_v), QK uses feature-major Q^T/K^T from 3 projection
slots [q0|q1], [k0|k1], [q2|k2] (k2 partition-shifted via SBUF DMA).
Causal masking touches only the [128,128] triangle block per diagonal
k-tile (gpsimd affine_select); the fully-masked columns are skipped by
column-restricted exp + PV accumulation. PV of group g is issued after
the QK+exp of group g+1 so the scalar engine (exp is the global floor,
~1.57us per k-tile) never starves.

Math per head (no max-subtraction softmax; scores are O(8) so exp is safe):
  S^T[k, q] = (K Q^T)[k, q] / 8     computed k-on-partitions (f32r matmuls)
  E = exp(S^T) * causal_mask
  [Y^T; l] = [V | 1]^T E            PV matmul with a ones column -> row 64 = l
  out += (Y^T / l).T @ W_o[head rows]
"""

import os
import numpy as np
import ml_dtypes
from contextlib import ExitStack

import concourse.bass as bass
import concourse.tile as tile
from concourse import bacc, mybir
from concourse.bass_utils import run_bass_kernel_spmd

F32 = mybir.dt.float32
BF16 = mybir.dt.bfloat16

B, T, C, H, D = 2, 4096, 768, 12, 64
HPC = 3            # heads per core
NS = 8             # strips
SW = 512           # strip width (q)
KT = 128           # k tile
NKT = T // KT      # 32 k tiles
KG = 8             # k tiles per PV accumulation group





def build_program():
    nc = bacc.Bacc("TRN2", target_bir_lowering=False, debug=False, num_devices=8)

    x_d = nc.dram_tensor("xT", [C, T], BF16, kind="ExternalInput").ap()
    wqk_d = nc.dram_tensor("wqk", [C, 384], BF16, kind="ExternalInput").ap()
    wv_d = nc.dram_tensor("wv", [C, 192], BF16, kind="ExternalInput").ap()
    woA_d = nc.dram_tensor("woA", [128, C], BF16, kind="ExternalInput").ap()
    woB_d = nc.dram_tensor("woB", [64, C], BF16, kind="ExternalInput").ap()
    y_d = nc.dram_tensor("y", [T, C], BF16, kind="ExternalOutput").ap()

    with tile.TileContext(nc) as tc, ExitStack() as ctx:
        kernel_body(tc, ctx, x_d, wqk_d, wv_d, woA_d, woB_d, y_d)
    nc.compile()
    return nc


def kernel_body(tc, ctx, x_d, wqk_d, wv_d, woA_d, woB_d, y_d):
    nc = tc.nc
    EXP = mybir.ActivationFunctionType.Exp
    k_diag = int(os.environ.get("KDIAG", "1"))   # col-restricted diag PV
    k_pb = int(os.environ.get("KPB", "0"))       # gpsimd partition_broadcast
    k_rf = int(os.environ.get("KRF", "0"))       # reciprocal_approx_fast
    k_warm = int(os.environ.get("KWARM", "1"))   # PE warmup matmuls
    k_keep = int(os.environ.get("KKEEP", "0"))   # HAM keepalive zero-MMs/tick
    k_keepfrom = int(os.environ.get("KKEEPFROM", "4"))
    dram_pool = ctx.enter_context(tc.tile_pool(name="dram", bufs=1, space="DRAM"))
    scratch_d = dram_pool.tile([NS, HPC, SW], F32, name="scratch")

    singles = ctx.enter_context(tc.tile_pool(name="singles", bufs=1))
    xt_pool = ctx.enter_context(tc.tile_pool(name="xt_pool", bufs=4))
    qq_pool = ctx.enter_context(tc.tile_pool(name="qq_pool", bufs=3))
    es_pool = ctx.enter_context(tc.tile_pool(name="es_pool", bufs=20))
    ya_pool = ctx.enter_context(tc.tile_pool(name="ya_pool", bufs=2))
    rl_pool = ctx.enter_context(tc.tile_pool(name="rl_pool", bufs=2))
    yst_pool = ctx.enter_context(tc.tile_pool(name="yst_pool", bufs=2))
    out_pool = ctx.enter_context(tc.tile_pool(name="out_pool", bufs=2))
    ps_s = ctx.enter_context(tc.tile_pool(name="ps_s", bufs=2, space="PSUM"))
    ps_y = ctx.enter_context(tc.tile_pool(name="ps_y", bufs=2, space="PSUM"))

    # ---- PE warmup: junk matmuls during the initial DMA wait keep HAM hot ----
    junk = singles.tile([128, 128], BF16)
    nc.vector.memset(junk, 0.015625)
    # zero stationary + junk moving: accumulating 0 into an open PV psum
    # tile is a numerically-free PE keepalive (HAM stays at K=8/8)
    junkz = singles.tile([128, D + 1], BF16)
    nc.vector.memset(junkz, 0.0)
    junkr = singles.tile([128, SW], BF16)
    nc.vector.memset(junkr, 0.0)
    ones64 = singles.tile([1, 64], BF16)
    nc.vector.memset(ones64, 1.0)
    if k_warm:
        psj = ps_y.tile([128, SW], F32, name="ps_warm", tag="psy")
        for w in range(32):
            nc.tensor.matmul(psj[:, 0:128], (junk), (junk),
                             start=True, stop=True)

    # ---- weights (xT strip 0 + wqk first: they gate the first QK) ----
    xT_tiles = [None] * NS
    x_r = x_d.rearrange("(kc p) t -> p kc t", kc=6)
    xt0 = xt_pool.tile([128, 6, SW], BF16, name="xT_0", tag="xT")
    for kc in range(6):
        eng = nc.sync if kc % 2 == 0 else nc.gpsimd
        eng.dma_start(xt0[:, kc, :], x_r[:, kc, 0:SW])
    xT_tiles[0] = xt0
    wqk_sb = singles.tile([128, 6, 384], BF16, name="wqk_sb")
    nc.sync.dma_start(wqk_sb, wqk_d.rearrange("(kc p) f -> p kc f", kc=6))
    wv_sb = singles.tile([128, 6, 192], BF16, name="wv_sb")
    nc.gpsimd.dma_start(wv_sb, wv_d.rearrange("(kc p) f -> p kc f", kc=6))
    woA = singles.tile([128, C], BF16)
    woB = singles.tile([64, C], BF16)

    # resident K storage: KK[s] = [k0|k1] feature-major, K2c[s] = k2 at p0:64
    KK = [singles.tile([128, SW], BF16, name=f"KK{s}") for s in range(NS)]
    K2c = [singles.tile([64, SW], BF16, name=f"K2c{s}") for s in range(NS)]

    # token-major V with ones column per head, all 32 k-tiles
    vtm = [singles.tile([128, NKT, D + 1], BF16, name=f"vtm{h}") for h in range(HPC)]
    ones_col = singles.tile([128, NKT], BF16)
    nc.vector.memset(ones_col, 1.0)
    for h in range(HPC):
        nc.vector.tensor_copy(vtm[h][:, :, D:D + 1], ones_col.unsqueeze(2))

    qq_tiles = [None] * NS

    # ---------------- Phase A for one strip (chunk generator) ----------------
    def phase_a_dma(s):
        xt = xt_pool.tile([128, 6, SW], BF16, name=f"xT_{s}", tag="xT")
        for kc in range(6):
            eng = nc.sync if kc % 2 == 0 else nc.gpsimd
            eng.dma_start(xt[:, kc, :], x_r[:, kc, s * SW:(s + 1) * SW])
        xT_tiles[s] = xt

    def phase_a_proj(s):
        xT = xT_tiles[s]
        # projection slots: [q0|q1], [k0|k1], [q2|k2] -- each slot gets its
        # own 1-bank psum tile from the psy ring so the QK double-buffer
        # ("S" ring) is never starved by projection work. Slot 2 first: its
        # k2 partition-shift DMA is on the critical path to the first QK.
        qq = tmp = None
        for u in (2, 0, 1):
            psp = ps_y.tile([128, SW], F32, name=f"ps_pj_{s}_{u}", tag="psy")
            for kc in range(6):
                nc.tensor.matmul(psp,
                                 (wqk_sb[:, kc, u * 128:(u + 1) * 128]),
                                 (xT[:, kc, :]), start=(kc == 0), stop=(kc == 5))
            if u == 0:
                qq = qq_pool.tile([128, SW], BF16, name=f"qq_{s}", tag="qq")
                nc.vector.tensor_copy(qq, psp)
                qq_tiles[s] = (qq, tmp)
            elif u == 1:
                nc.vector.tensor_copy(KK[s], psp)
            else:
                tmp = qq_pool.tile([128, SW], BF16, name=f"q2k2_{s}",
                                   tag="q2k2")
                nc.vector.tensor_copy(tmp, psp)
                # k2 partition shift p64:128 -> p0:64 (SBUF->SBUF DMA)
                nc.gpsimd.dma_start(K2c[s], tmp[64:128, :])
            yield

    def phase_a_v(s):
        # V token-major: stationary = xT chunk slice, moving = w_v [128, 192]
        # (pure filler: vtm k-tiles of strip s are first read by strip s's
        # diagonal PV unit, which issues at the end of strip s)
        xT = xT_tiles[s]
        for tt in range(4):
            psv = ps_y.tile([128, 192], F32, name=f"ps_v_{s}_{tt}", tag="psy")
            for kc in range(6):
                nc.tensor.matmul(psv,
                                 (xT[:, kc, tt * 128:(tt + 1) * 128]),
                                 (wv_sb[:, kc, :]), start=(kc == 0), stop=(kc == 5))
            kt = 4 * s + tt
            for h in range(HPC):
                nc.vector.tensor_copy(vtm[h][:, kt, 0:D],
                                      psv[:, h * 64:(h + 1) * 64])
            yield

    # -------- Phase B: one continuous pipeline over all 144 k-tiles --------
    # Per tick (one k-tile): QK triplet + exp + ~3 PV matmuls from the unit
    # queue (one (strip, group, head) unit at a time, so only 1-2 psy banks
    # are ever live) + one filler chunk. PV lags its group's last exp by
    # >= 2 ticks so the PE FIFO never blocks on the scalar engine.
    fillers = []
    pa_gens = {}

    def fill_one():
        while fillers:
            g = fillers.pop(0)
            try:
                next(g)
            except StopIteration:
                continue
            fillers.append(g)
            return True
        return False

    yaccs = {}

    open_psy = {"t": None}

    def make_unit(s, gi, grp, h, es_grp):
        """Returns list of thunks: 8 (or 4) PV matmuls then the yacc flush."""
        psy_box = {}

        def mm(u, i):
            def run():
                if u == 0:
                    psy_box["t"] = ps_y.tile([65, SW], F32,
                                             name=f"ps_y_{s}_{gi}_{h}", tag="psy")
                    open_psy["t"] = psy_box["t"]
                psy = psy_box["t"]
                es = es_grp[i]
                o = i - 4 * s
                last = len(grp) - 1
                if k_diag and o > 0:
                    nc.tensor.matmul(psy[:, 128 * o:], (vtm[h][:, i, :]),
                                     (es[:, h, 128 * o:]),
                                     start=False, stop=(u == last))
                else:
                    nc.tensor.matmul(psy, (vtm[h][:, i, :]), (es[:, h, :]),
                                     start=(u == 0), stop=(u == last))
                if u == last and open_psy["t"] is psy:
                    open_psy["t"] = None
            return run

        def flush():
            psy = psy_box["t"]
            if open_psy["t"] is psy:
                open_psy["t"] = None
            if gi == 0:
                nc.vector.tensor_copy(yaccs[s][h], psy)
            else:
                nc.vector.tensor_add(yaccs[s][h], yaccs[s][h], psy)

        thunks = [mm(u, i) for u, i in enumerate(grp)]
        thunks.append(flush)
        return thunks

    # PV work queue: per tick pop up to 3 thunks whose eligibility tick passed
    pvq = []          # list of (eligible_tick, thunk)

    def pump_pv(tick, n=3):
        done = 0
        while pvq and done < n:
            et, th = pvq[0]
            if et > tick:
                break
            pvq.pop(0)
            th()
            if th.__name__ != "flush":
                done += 1

    def run_pipeline():
        tick = 0
        for s in range(NS):
            nkt = 4 * s + 4
            if s == 1:
                nc.sync.dma_start(woA, woA_d)
                nc.sync.dma_start(woB, woB_d)
            if s + 2 < NS:
                phase_a_dma(s + 2)
            if s + 1 < NS:
                g = phase_a_proj(s + 1)
                pa_gens[s + 1] = g
                fillers.append(g)
            fillers.append(phase_a_v(s))
            # ensure this strip's projections are fully issued
            g = pa_gens.get(s)
            if g is not None:
                for _ in g:
                    pass
            qq, tmp = qq_tiles[s]
            qq2 = tmp[0:64, :]
            yaccs[s] = [ya_pool.tile([65, SW], F32, name=f"yacc_{s}_{h}",
                                     tag=f"yacc{h}") for h in range(HPC)]

            groups = [list(range(gg, min(gg + KG, nkt)))
                      for gg in range(0, nkt, KG)]
            es_grp = {}
            for gi, grp in enumerate(groups):
                for u, i in enumerate(grp):
                    pss = ps_s.tile([128, 3, SW], F32,
                                    name=f"ps_s_{s}_{i}", tag="S")
                    st = KK[i // 4]
                    sl = slice((i % 4) * 128, (i % 4) * 128 + 128)
                    nc.tensor.matmul(pss[:, 0, :], (st[0:64, sl]),
                                     (qq[0:64, :]), start=True, stop=True)
                    nc.tensor.matmul(pss[:, 1, :], (st[64:128, sl]),
                                     (qq[64:128, :]), start=True, stop=True)
                    nc.tensor.matmul(pss[:, 2, :], (K2c[i // 4][:, sl]),
                                     (qq2), start=True, stop=True)
                    es = es_pool.tile([128, 3, SW], BF16,
                                      name=f"es_{s}_{i}", tag="es")
                    o = i - 4 * s
                    if o < 0:
                        nc.scalar.activation(es, pss, EXP, scale=0.125)
                    else:
                        nc.scalar.activation(es[:, :, 128 * o:],
                                             pss[:, :, 128 * o:],
                                             EXP, scale=0.125)
                        for h in range(HPC):
                            blk = es[:, h, 128 * o:128 * (o + 1)]
                            nc.gpsimd.affine_select(
                                out=blk, in_=blk,
                                compare_op=mybir.AluOpType.is_ge, fill=0.0,
                                base=0, pattern=[[1, 128]],
                                channel_multiplier=-1)
                        if not k_diag and o > 0:
                            nc.gpsimd.memset(es[:, :, 0:128 * o], 0.0)
                    es_grp[i] = es
                    pump_pv(tick, n=4 if s == NS - 1 else 3)
                    if k_keep and s >= k_keepfrom and open_psy["t"] is not None:
                        for _ in range(k_keep):
                            nc.tensor.matmul(open_psy["t"], (junkz), (junkr),
                                             start=False, stop=False)
                    fill_one()
                    if 1 <= s <= 2:
                        fill_one()
                    tick += 1
                # group's exps all issued: enqueue its 3 PV units
                et = tick + 1
                for h in range(HPC):
                    for th in make_unit(s, gi, grp, h, dict(es_grp)):
                        pvq.append((et, th))
            # strip done: schedule epilogue after its last units complete
            if s > 0:
                fillers.append(epilogue(s - 1, yaccs[s - 1]))
        # drain
        while pvq:
            et, th = pvq.pop(0)
            th()
            fill_one()
        for _ in epilogue(NS - 1, yaccs[NS - 1]):
            pass
        while fill_one():
            pass

    # ---- strip epilogue: normalize + output projection (deferred) ----
    # Fully chunked per 128 q-columns: reciprocal, DRAM-bounce broadcast,
    # normalize and output projection pipeline across DVE/DMA/PE so the
    # final strip's tail is short.
    def epilogue(s, yacc):
        lrow = rl_pool.tile([3, SW], F32, name=f"lrow_{s}", tag="lrow")
        for h in range(HPC):
            eng = nc.sync if h == 0 else nc.gpsimd
            eng.dma_start(lrow[h:h + 1, :], yacc[h][64:65, :])
        yield
        for ch in range(4):
            csl = slice(ch * 128, (ch + 1) * 128)
            nc.vector.reciprocal(lrow[:, csl], lrow[:, csl])
            nc.sync.dma_start(scratch_d[s, :, csl], lrow[:, csl])
            yield

        ya = yst_pool.tile([128, SW], BF16, name=f"ya_{s}", tag="ya")
        y2 = yst_pool.tile([64, SW], BF16, name=f"y2_{s}", tag="y2")
        ytmp = yst_pool.tile([64, SW], BF16, name=f"ytmp_{s}", tag="ytmp")
        for ch in range(4):
            csl = slice(ch * 128, (ch + 1) * 128)
            rb = rl_pool.tile([64, 3, 128], F32, name=f"rlb_{s}_{ch}",
                              tag="rlb")
            nc.gpsimd.dma_start(
                rb, scratch_d[s, :, csl].unsqueeze(0).to_broadcast((64, 3, 128)))
            nc.vector.tensor_mul(ya[0:64, csl], yacc[0][0:64, csl], rb[:, 0, :])
            nc.vector.tensor_mul(ytmp[:, csl], yacc[1][0:64, csl], rb[:, 1, :])
            nc.vector.tensor_mul(y2[:, csl], yacc[2][0:64, csl], rb[:, 2, :])
            nc.gpsimd.dma_start(ya[64:128, csl], ytmp[:, csl])
            yield

            # out projection for this 128-q tile
            qsl = csl
            osb = out_pool.tile([128, C], BF16, name=f"osb_{s}_{ch}", tag="osb")
            for (n0, n1) in ((0, 512), (512, 768)):
                pso = ps_y.tile([128, n1 - n0], F32,
                                name=f"ps_o_{s}_{ch}_{n0}", tag="psy")
                nc.tensor.matmul(pso, (ya[:, qsl]),
                                 (woA[:, n0:n1]), start=True, stop=False)
                nc.tensor.matmul(pso, (y2[:, qsl]),
                                 (woB[:, n0:n1]), start=False, stop=True)
                nc.vector.tensor_copy(osb[:, n0:n1], pso)
            nc.sync.dma_start(y_d[s * SW + ch * 128: s * SW + (ch + 1) * 128, :],
                              osb)
            if ch < 3:
                yield

    for _ in phase_a_proj(0):
        pass
    phase_a_dma(1)
    run_pipeline()


_PROGRAM_CACHE = {}


def _get_program():
    if "nc" not in _PROGRAM_CACHE:
        _PROGRAM_CACHE["nc"] = build_program()
    return _PROGRAM_CACHE["nc"]


def make_in_maps(x, w_qkv, w_out):
    x = np.asarray(x, dtype=np.float32)
    w_qkv = np.asarray(w_qkv, dtype=np.float32)
    w_out = np.asarray(w_out, dtype=np.float32)
    in_maps = []
    for c in range(8):
        b, g = c // 4, c % 4
        base = 192 * g
        q01 = w_qkv[:, base:base + 128]
        q2 = w_qkv[:, base + 128:base + 192]
        k01 = w_qkv[:, 768 + base:768 + base + 128]
        k2 = w_qkv[:, 768 + base + 128:768 + base + 192]
        wqk = np.concatenate([q01, k01, q2, k2], axis=1)
        wv = w_qkv[:, 1536 + base:1536 + base + 192]
        bf = ml_dtypes.bfloat16
        in_maps.append({
            "xT": np.ascontiguousarray(x[b].T.astype(bf)),
            "wqk": np.ascontiguousarray(wqk.astype(bf)),
            "wv": np.ascontiguousarray(wv.astype(bf)),
            "woA": np.ascontiguousarray(w_out[base:base + 128].astype(bf)),
            "woB": np.ascontiguousarray(w_out[base + 128:base + 192].astype(bf)),
        })
    return in_maps


def kernel(x, w_qkv, w_out, trace=False):
    nc = _get_program()
    in_maps = make_in_maps(x, w_qkv, w_out)
    res = run_bass_kernel_spmd(nc, in_maps, list(range(8)), trace=trace)
    out = np.zeros((B, T, C), dtype=np.float32)
    for c in range(8):
        out[c // 4] += np.asarray(res.results[c]["y"], dtype=np.float32)
    kernel.last_result = res
    return out
